# revision 11
# baseline (speedup 1.0000x reference)
"""Trainium2 Bass kernel for nn_DendriteOutput.

Math: out[b, o] = sum_{d<32} x[b, o*32+d] * weight[o, o*32+d] + bias[o]
(block-diagonal connectivity: only the diagonal 32-wide blocks of `weight`
are touched; the other 99.2% of the matrix is never read).

Sharding (8 cores, tensor-parallel over out_dim): core k owns outputs
[k*256, (k+1)*256) for the full batch, i.e. the x column-slab
[:, k*8192:(k+1)*8192].

Host-side layout (this is the sharding layer, done in numpy):
  * x slab is transposed to feature-major [8192, 1024] and cast to fp16 --
    halves the dominant HBM stream (16 MB/core instead of 32 MB).
  * the 256 diagonal 32-wide weight blocks are packed into 64 sparse
    "lhsT" chunks of [K=128, M=32] fp16: chunk c covers features
    [128c, 128c+128) (= outputs [4c, 4c+4)); column m = 4*(c%8)+j holds
    weight[4c+j, :] at partitions 32j..32j+32.  512 KB total.
  * bias (fp16) and a ones-row are packed into one [1, 768] blob.

Device pipeline per core (pure TensorE compute):
  * HWDGE DMAs stream x_t in 1 MB tiles [128 feat, 4 chunks x 1024 batch].
  * For each 128-output block and 512-batch half, a PSUM bank [128, 512]
    is seeded with bias via a rank-1 matmul (lhsT = bias row, rhs = ones,
    start=True -> clears the bank and writes bias everywhere), then 32
    chunk matmuls (K=128, M=32, start=False) accumulate the block-diagonal
    dot products at 32-aligned psum partition offsets (auto tile_position
    (0, 32g)).
  * ScalarE evacuates PSUM -> SBUF fp16, HWDGE stores y_t [256, 1024].
Host transposes y_t back to [1024, 256] per core and concatenates.

Engine budget per core per rep (HW-measured via A/B ablations): DMA
16.5 MB at ~378 GB/s/core = ~43.5 us (dma_only ablation == full kernel,
i.e. compute is fully hidden and the stream saturates the chip's
aggregate HBM bandwidth across all 8 cores); PE span ~18.5 us; ScalarE
~3 us; DVE 0.  fp8 x would halve the stream but its ~3.6% quantization
error fails the 2e-2 gate, so fp16 is the byte floor.

All reps of the timing harness live in ONE TileContext so reps pipeline
without per-context drain barriers; the (tiny) weight/bias blobs are
loaded once and stay resident in SBUF across reps, while the full x
stream + y store traffic is repeated every rep.
"""

import json

import numpy as np

import concourse.bass as bass
import concourse.bass_utils as _bass_utils
import concourse.mybir as mybir
from concourse.tile import TileContext
from concourse.bass_utils import run_bass_kernel_spmd

BATCH = 1024
OUT_DIM = 2048
DPC = 32
N_CORES = 8
O_PER = OUT_DIM // N_CORES          # 256 outputs per core
F_PER = O_PER * DPC                 # 8192 features per core
N_CHUNK = F_PER // 128              # 64 lhsT chunks per core
CPT = 4                             # chunks per x tile (1 MB tiles)
XB = 4                              # x tile double-buffer depth
PSB = 4                             # psum pool depth (banks)
MODE = "full"                       # full | dma_only | pe_only (perf triage)
LOADQ = "sync"                      # sync | alt (alternate HWDGE queues)

# ---------------------------------------------------------------------------
# Environment workarounds (in-process only; nothing on disk is modified).
#
# The walrus build in this container (a) needs --dge-levels to lower HWDGE
# DMAs with sem waits (otherwise they hit the V2 pseudo-DMA path that allows
# none) and (b) caps sync waits at ONE per instruction while Tile attaches up
# to N (e.g. the kernel-tail drain). We add the flag and rewrite the
# serialized BIR: extra waits are hoisted into preceding single-wait Drain
# carriers on the same engine (safe: a wait only moves earlier within the
# same engine-program order).
# ---------------------------------------------------------------------------

_patched = False


def _patch_walrus_flags():
    global _patched
    if _patched:
        return
    _patched = True
    orig_rc = _bass_utils.run_command

    def rc(cmd, cwd=None, **kw):
        if cmd and "walrus_driver" in str(cmd[0]):
            cmd = list(cmd)
            cmd.insert(1, "--dge-levels=io,spill_reload,scalar_dynamic_offset")
        return orig_rc(cmd, cwd=cwd, **kw)

    _bass_utils.run_command = rc


def _split_multi_waits(bir_bytes: bytes, cap: int = 1) -> bytes:
    m = json.loads(bir_bytes)
    for fn in m["functions"]:
        for blk in fn["blocks"]:
            out = []
            for inst in blk["instructions"]:
                si = inst.get("sync_info")
                waits = (si or {}).get("on_wait") or []
                if len(waits) > cap:
                    keep = waits[-cap:]
                    for j, wchunk in enumerate(waits[:-cap]):
                        out.append(
                            {
                                "debug": inst.get("debug"),
                                "engine": inst["engine"],
                                "ins": [],
                                "name": f"{inst['name']}-ws{j}",
                                "opcode": "Drain",
                                "outs": [],
                                "sync_info": {
                                    "on_update": [],
                                    "on_wait": [wchunk],
                                },
                            }
                        )
                    si["on_wait"] = keep
                out.append(inst)
            blk["instructions"] = out
    return json.dumps(m).encode()


def _emit_rep(nc, xpool, ppool, ypool, wl_sb, bb_sb, x, y, last_rep):
    fp16 = mybir.dt.float16
    f32 = mybir.dt.float32
    ones = bb_sb[0:1, O_PER:O_PER + 512]
    n_t = N_CHUNK // 2 // CPT                     # x-tiles per m-block
    for mb in range(2):
        if MODE != "dma_only":
            ps = [ppool.tile([128, 512], f32, tag="ps", name=f"ps{mb}_{bh}")
                  for bh in range(2)]
            bl = bb_sb[0:1, mb * 128:(mb + 1) * 128]
            for bh in range(2):
                # Rank-1 bias seed: clears the bank (start=True) and writes
                # bias[m] to every column, setting has_written everywhere so
                # the chunk matmuls below accumulate onto it.
                nc.tensor.matmul(ps[bh][:, :], bl, ones,
                                 start=True, stop=False, skip_group_check=True,
                                 tile_position=(0, 0))
        ysb = ypool.tile([128, 1024], fp16, tag="ysb")
        for t in range(n_t):
            xt = xpool.tile([128, CPT * 1024], fp16, tag="xt")
            xv = xt[:].rearrange("p (c b) -> p c b", b=1024)
            c0 = mb * 32 + t * CPT
            # x is staged k-major on host: x[k, c, b]; partition k reads a
            # contiguous CPT*2KB run per tile.
            eng = nc.sync if (LOADQ == "sync" or t % 2 == 0) else nc.scalar
            if MODE == "pe_only":
                eng.dma_start(xv[:, 0, 0:64],
                              bass.AP(x, c0 * 1024,
                                      [[N_CHUNK * 1024, 128], [1, 64]]))
            else:
                src = bass.AP(x, c0 * 1024,
                              [[N_CHUNK * 1024, 128], [1024, CPT], [1, 1024]])
                eng.dma_start(xv, src)
            if MODE == "dma_only":
                continue
            for cs in range(CPT):
                cg = c0 + cs                      # global chunk 0..63
                cl = cg - mb * 32                 # chunk within m-block
                g = cl // 8                       # 32-row psum group
                lv = wl_sb[:, cg * 32:(cg + 1) * 32]
                last = (t == n_t - 1 and cs == CPT - 1)
                for bh in range(2):
                    nc.tensor.matmul(
                        ps[bh][32 * g:32 * (g + 1), :], lv,
                        xv[:, cs, bh * 512:(bh + 1) * 512],
                        start=False, stop=last, skip_group_check=True,
                        tile_position=(0, 32 * g))
        if MODE == "dma_only":
            nc.scalar.dma_start(y[mb * 128:(mb + 1) * 128, :],
                                wl_sb[:, 0:1024])
            continue
        for bh in range(2):
            nc.scalar.copy(ysb[:, bh * 512:(bh + 1) * 512], ps[bh][:, :])
        nc.scalar.dma_start(y[mb * 128:(mb + 1) * 128, :], ysb[:])


def _build_program(n_reps=1):
    fp16 = mybir.dt.float16
    nc = bass.Bass()
    x = nc.dram_tensor("x", [128, N_CHUNK * BATCH], fp16, kind="ExternalInput")
    wl = nc.dram_tensor("wl", [128, N_CHUNK * 32], fp16, kind="ExternalInput")
    bb = nc.dram_tensor("bb", [1, O_PER + 512], fp16, kind="ExternalInput")
    y = nc.dram_tensor("y", [O_PER, BATCH], fp16, kind="ExternalOutput")
    with TileContext(nc) as tc:
        with tc.tile_pool(name="const", bufs=1) as cpool, \
             tc.tile_pool(name="xp", bufs=XB) as xpool, \
             tc.tile_pool(name="pp", bufs=PSB, space="PSUM") as ppool, \
             tc.tile_pool(name="yp", bufs=3) as ypool:
            wl_sb = cpool.tile([128, N_CHUNK * 32], fp16, name="wl_sb")
            bb_sb = cpool.tile([1, O_PER + 512], fp16, name="bb_sb")
            nc.scalar.dma_start(wl_sb[:], wl[:, :])
            nc.scalar.dma_start(bb_sb[:], bb[:, :])
            for rep in range(n_reps):
                _emit_rep(nc, xpool, ppool, ypool, wl_sb, bb_sb, x, y,
                          last_rep=(rep == n_reps - 1))
    return nc


def _finalize(nc):
    data = _split_multi_waits(nc.to_json_bytes())
    nc.to_json_bytes = lambda: data
    return nc


_CACHED = None


def _get_program():
    global _CACHED
    if _CACHED is None:
        _patch_walrus_flags()
        _CACHED = _finalize(_build_program())
    return _CACHED


def _shard_inputs(x, weight, bias):
    x = np.asarray(x, dtype=np.float32)
    weight = np.asarray(weight, dtype=np.float32)
    bias = np.asarray(bias, dtype=np.float32)
    assert x.shape == (BATCH, OUT_DIM * DPC)
    assert weight.shape == (OUT_DIM, OUT_DIM * DPC)
    c_idx = np.arange(N_CHUNK)
    in_maps = []
    for k in range(N_CORES):
        fs = slice(k * F_PER, (k + 1) * F_PER)
        os_ = slice(k * O_PER, (k + 1) * O_PER)
        # k-major staging: xt[k, c, b] = x[b, fs][c*128 + k] so each SBUF
        # partition k streams contiguous HBM per tile.
        xt = np.ascontiguousarray(
            x[:, fs].T.astype(np.float16)
            .reshape(N_CHUNK, 128, BATCH).transpose(1, 0, 2)
        ).reshape(128, N_CHUNK * BATCH)
        # Diagonal 32-blocks of this core's weight slab: wd[o, d]
        wb = weight[os_, fs].reshape(O_PER, O_PER, DPC)
        wd = wb[np.arange(O_PER), np.arange(O_PER)]          # [256, 32] f32
        # Pack sparse lhsT chunks: wl[32*j+d, c, 4*(c%8)+j] = wd[4c+j, d]
        wlk = np.zeros((4, DPC, N_CHUNK, 32), np.float16)
        m_base = 4 * (c_idx % 8)
        for j in range(4):
            wlk[j, :, c_idx, m_base + j] = wd[4 * c_idx + j].astype(np.float16)
        wlk = wlk.reshape(128, N_CHUNK * 32)
        bbk = np.zeros((1, O_PER + 512), np.float16)
        bbk[0, :O_PER] = bias[os_].astype(np.float16)
        bbk[0, O_PER:] = np.float16(1.0)
        in_maps.append({"x": xt, "wl": np.ascontiguousarray(wlk), "bb": bbk})
    return in_maps


def kernel(x, weight, bias):
    nc = _get_program()
    in_maps = _shard_inputs(x, weight, bias)
    res = run_bass_kernel_spmd(nc, in_maps, list(range(N_CORES))).results
    out = np.empty((BATCH, OUT_DIM), np.float32)
    for k in range(N_CORES):
        out[:, k * O_PER:(k + 1) * O_PER] = res[k]["y"].T.astype(np.float32)
    return out


if __name__ == "__main__":
    rng = np.random.default_rng(0)
    x = rng.standard_normal((BATCH, OUT_DIM * DPC), dtype=np.float32)
    w = rng.standard_normal((OUT_DIM, OUT_DIM * DPC), dtype=np.float32)
    b_ = rng.standard_normal(OUT_DIM).astype(np.float32)
    out = kernel(x, w, b_)
    xb = x.reshape(BATCH, OUT_DIM, DPC)
    wb = np.stack([w[o, o * DPC: (o + 1) * DPC] for o in range(OUT_DIM)])
    exp = np.einsum("bod,od->bo", xb, wb) + b_
    rel = np.linalg.norm(out - exp) / np.linalg.norm(exp)
    print("rel err:", rel)


# revision 16
# speedup vs baseline: 1.2290x; 1.2290x over previous
"""Trainium2 Bass kernel for nn_DendriteOutput.

Math: out[b, o] = sum_{d<32} x[b, o*32+d] * weight[o, o*32+d] + bias[o]
(block-diagonal connectivity: only the diagonal 32-wide blocks of `weight`
are touched; the other 99.2% of the matrix is never read).

Sharding (8 cores, tensor-parallel over out_dim): core k owns outputs
[k*256, (k+1)*256) for the full batch, i.e. the x column-slab
[:, k*8192:(k+1)*8192].

Host-side layout (this is the sharding layer, done in numpy):
  * x slab is transposed to feature-major [8192, 1024] and cast to fp16 --
    halves the dominant HBM stream (16 MB/core instead of 32 MB).
  * the 256 diagonal 32-wide weight blocks are packed into 64 sparse
    "lhsT" chunks of [K=128, M=32] fp16: chunk c covers features
    [128c, 128c+128) (= outputs [4c, 4c+4)); column m = 4*(c%8)+j holds
    weight[4c+j, :] at partitions 32j..32j+32.  512 KB total.
  * bias (fp16) and a ones-row are packed into one [1, 768] blob.

Device pipeline per core (pure TensorE compute):
  * HWDGE DMAs stream x_t in 1 MB tiles [128 feat, 4 chunks x 1024 batch].
  * For each 128-output block and 512-batch half, a PSUM bank [128, 512]
    is seeded with bias via a rank-1 matmul (lhsT = bias row, rhs = ones,
    start=True -> clears the bank and writes bias everywhere), then 32
    chunk matmuls (K=128, M=32, start=False) accumulate the block-diagonal
    dot products at 32-aligned psum partition offsets (auto tile_position
    (0, 32g)).
  * ScalarE evacuates PSUM -> SBUF fp16, HWDGE stores y_t [256, 1024].
Host transposes y_t back to [1024, 256] per core and concatenates.

Engine budget per core per rep (HW-measured via A/B ablations): DMA
16.5 MB at ~378 GB/s/core = ~43.5 us (dma_only ablation == full kernel,
i.e. compute is fully hidden and the stream saturates the chip's
aggregate HBM bandwidth across all 8 cores); PE span ~18.5 us; ScalarE
~3 us; DVE 0.  fp8 x would halve the stream but its ~3.6% quantization
error fails the 2e-2 gate, so fp16 is the byte floor.

All reps of the timing harness live in ONE TileContext so reps pipeline
without per-context drain barriers; the (tiny) weight/bias blobs are
loaded once and stay resident in SBUF across reps, while the full x
stream + y store traffic is repeated every rep.
"""

import json

import numpy as np

import concourse.bass as bass
import concourse.bass_utils as _bass_utils
import concourse.mybir as mybir
from concourse.tile import TileContext
from concourse.bass_utils import run_bass_kernel_spmd

BATCH = 1024
OUT_DIM = 2048
DPC = 32
N_CORES = 8
O_PER = OUT_DIM // N_CORES          # 256 outputs per core
F_PER = O_PER * DPC                 # 8192 features per core
N_CHUNK = F_PER // 128              # 64 lhsT chunks per core
CPT = 4                             # chunks per x tile (1 MB tiles)
XB = 4                              # x tile double-buffer depth
PSB = 4                             # psum pool depth (banks)
MODE = "full"                       # full | dma_only | pe_only (perf triage)
LOADQ = "sync"                      # sync | alt (alternate HWDGE queues)
TC = 0                              # 1: per-tile-contiguous HBM layout
                                    # (each x tile = one linear CPT*256KB
                                    # span); 0: core-slab k-major layout

# ---------------------------------------------------------------------------
# Environment workarounds (in-process only; nothing on disk is modified).
#
# The walrus build in this container (a) needs --dge-levels to lower HWDGE
# DMAs with sem waits (otherwise they hit the V2 pseudo-DMA path that allows
# none) and (b) caps sync waits at ONE per instruction while Tile attaches up
# to N (e.g. the kernel-tail drain). We add the flag and rewrite the
# serialized BIR: extra waits are hoisted into preceding single-wait Drain
# carriers on the same engine (safe: a wait only moves earlier within the
# same engine-program order).
# ---------------------------------------------------------------------------

_patched = False


def _patch_walrus_flags():
    global _patched
    if _patched:
        return
    _patched = True
    orig_rc = _bass_utils.run_command

    def rc(cmd, cwd=None, **kw):
        if cmd and "walrus_driver" in str(cmd[0]):
            cmd = list(cmd)
            cmd.insert(1, "--dge-levels=io,spill_reload,scalar_dynamic_offset")
        return orig_rc(cmd, cwd=cwd, **kw)

    _bass_utils.run_command = rc


def _split_multi_waits(bir_bytes: bytes, cap: int = 1) -> bytes:
    m = json.loads(bir_bytes)
    for fn in m["functions"]:
        for blk in fn["blocks"]:
            out = []
            for inst in blk["instructions"]:
                si = inst.get("sync_info")
                waits = (si or {}).get("on_wait") or []
                if len(waits) > cap:
                    keep = waits[-cap:]
                    for j, wchunk in enumerate(waits[:-cap]):
                        out.append(
                            {
                                "debug": inst.get("debug"),
                                "engine": inst["engine"],
                                "ins": [],
                                "name": f"{inst['name']}-ws{j}",
                                "opcode": "Drain",
                                "outs": [],
                                "sync_info": {
                                    "on_update": [],
                                    "on_wait": [wchunk],
                                },
                            }
                        )
                    si["on_wait"] = keep
                out.append(inst)
            blk["instructions"] = out
    return json.dumps(m).encode()


def _emit_rep(nc, xpool, ppool, ypool, wl_sb, bb_sb, x, y, last_rep):
    fp16 = mybir.dt.float16
    f32 = mybir.dt.float32
    ones = bb_sb[0:1, O_PER:O_PER + 512]
    n_t = N_CHUNK // 2 // CPT                     # x-tiles per m-block
    for mb in range(2):
        if MODE != "dma_only":
            ps = [ppool.tile([128, 512], f32, tag="ps", name=f"ps{mb}_{bh}")
                  for bh in range(2)]
            bl = bb_sb[0:1, mb * 128:(mb + 1) * 128]
            for bh in range(2):
                # Rank-1 bias seed: clears the bank (start=True) and writes
                # bias[m] to every column, setting has_written everywhere so
                # the chunk matmuls below accumulate onto it.
                nc.tensor.matmul(ps[bh][:, :], bl, ones,
                                 start=True, stop=False, skip_group_check=True,
                                 tile_position=(0, 0))
        ysb = ypool.tile([128, 1024], fp16, tag="ysb")
        for t in range(n_t):
            xt = xpool.tile([128, CPT * 1024], fp16, tag="xt")
            xv = xt[:].rearrange("p (c b) -> p c b", b=1024)
            c0 = mb * 32 + t * CPT
            # x is staged k-major on host: x[k, c, b]; partition k reads a
            # contiguous CPT*2KB run per tile.
            eng = nc.sync if (LOADQ == "sync" or t % 2 == 0) else nc.scalar
            if TC:
                tg = c0 // CPT                # global tile index
                off = tg * 128 * CPT * 1024
                strides = [[CPT * 1024, 128], [1024, CPT], [1, 1024]]
            else:
                off = c0 * 1024
                strides = [[N_CHUNK * 1024, 128], [1024, CPT], [1, 1024]]
            if MODE == "pe_only":
                eng.dma_start(xv[:, 0, 0:64],
                              bass.AP(x, off, [strides[0], [1, 64]]))
            else:
                eng.dma_start(xv, bass.AP(x, off, strides))
            if MODE == "dma_only":
                continue
            for cs in range(CPT):
                cg = c0 + cs                      # global chunk 0..63
                cl = cg - mb * 32                 # chunk within m-block
                g = cl // 8                       # 32-row psum group
                lv = wl_sb[:, cg * 32:(cg + 1) * 32]
                last = (t == n_t - 1 and cs == CPT - 1)
                for bh in range(2):
                    nc.tensor.matmul(
                        ps[bh][32 * g:32 * (g + 1), :], lv,
                        xv[:, cs, bh * 512:(bh + 1) * 512],
                        start=False, stop=last, skip_group_check=True,
                        tile_position=(0, 32 * g))
        if MODE == "dma_only":
            nc.scalar.dma_start(y[mb * 128:(mb + 1) * 128, :],
                                wl_sb[:, 0:1024])
            continue
        for bh in range(2):
            nc.scalar.copy(ysb[:, bh * 512:(bh + 1) * 512], ps[bh][:, :])
        nc.scalar.dma_start(y[mb * 128:(mb + 1) * 128, :], ysb[:])


def _build_program(n_reps=1):
    fp16 = mybir.dt.float16
    nc = bass.Bass()
    if TC:
        x = nc.dram_tensor("x", [N_CHUNK // CPT * 128, CPT * BATCH], fp16,
                           kind="ExternalInput")
    else:
        x = nc.dram_tensor("x", [128, N_CHUNK * BATCH], fp16,
                           kind="ExternalInput")
    wl = nc.dram_tensor("wl", [128, N_CHUNK * 32], fp16, kind="ExternalInput")
    bb = nc.dram_tensor("bb", [1, O_PER + 512], fp16, kind="ExternalInput")
    y = nc.dram_tensor("y", [O_PER, BATCH], fp16, kind="ExternalOutput")
    with TileContext(nc) as tc:
        with tc.tile_pool(name="const", bufs=1) as cpool, \
             tc.tile_pool(name="xp", bufs=XB) as xpool, \
             tc.tile_pool(name="pp", bufs=PSB, space="PSUM") as ppool, \
             tc.tile_pool(name="yp", bufs=3) as ypool:
            wl_sb = cpool.tile([128, N_CHUNK * 32], fp16, name="wl_sb")
            bb_sb = cpool.tile([1, O_PER + 512], fp16, name="bb_sb")
            nc.scalar.dma_start(wl_sb[:], wl[:, :])
            nc.scalar.dma_start(bb_sb[:], bb[:, :])
            for rep in range(n_reps):
                _emit_rep(nc, xpool, ppool, ypool, wl_sb, bb_sb, x, y,
                          last_rep=(rep == n_reps - 1))
    return nc


def _finalize(nc):
    data = _split_multi_waits(nc.to_json_bytes())
    nc.to_json_bytes = lambda: data
    return nc


_CACHED = None


def _get_program():
    global _CACHED
    if _CACHED is None:
        _patch_walrus_flags()
        _CACHED = _finalize(_build_program())
    return _CACHED


def _shard_inputs(x, weight, bias):
    x = np.asarray(x, dtype=np.float32)
    weight = np.asarray(weight, dtype=np.float32)
    bias = np.asarray(bias, dtype=np.float32)
    assert x.shape == (BATCH, OUT_DIM * DPC)
    assert weight.shape == (OUT_DIM, OUT_DIM * DPC)
    c_idx = np.arange(N_CHUNK)
    in_maps = []
    for k in range(N_CORES):
        fs = slice(k * F_PER, (k + 1) * F_PER)
        os_ = slice(k * O_PER, (k + 1) * O_PER)
        xf = x[:, fs].T.astype(np.float16)          # [8192, 1024] feat-major
        if TC:
            # Per-tile-contiguous staging: tile t (CPT chunks) is one linear
            # HBM span, k-major inside so each partition reads one CPT*2KB run.
            n_t = N_CHUNK // CPT
            xt = np.ascontiguousarray(
                xf.reshape(n_t, CPT, 128, BATCH).transpose(0, 2, 1, 3)
            ).reshape(n_t * 128, CPT * BATCH)
        else:
            # Core-slab k-major staging: xt[k, c, b] = x[b, fs][c*128 + k].
            xt = np.ascontiguousarray(
                xf.reshape(N_CHUNK, 128, BATCH).transpose(1, 0, 2)
            ).reshape(128, N_CHUNK * BATCH)
        # Diagonal 32-blocks of this core's weight slab: wd[o, d]
        wb = weight[os_, fs].reshape(O_PER, O_PER, DPC)
        wd = wb[np.arange(O_PER), np.arange(O_PER)]          # [256, 32] f32
        # Pack sparse lhsT chunks: wl[32*j+d, c, 4*(c%8)+j] = wd[4c+j, d]
        wlk = np.zeros((4, DPC, N_CHUNK, 32), np.float16)
        m_base = 4 * (c_idx % 8)
        for j in range(4):
            wlk[j, :, c_idx, m_base + j] = wd[4 * c_idx + j].astype(np.float16)
        wlk = wlk.reshape(128, N_CHUNK * 32)
        bbk = np.zeros((1, O_PER + 512), np.float16)
        bbk[0, :O_PER] = bias[os_].astype(np.float16)
        bbk[0, O_PER:] = np.float16(1.0)
        in_maps.append({"x": xt, "wl": np.ascontiguousarray(wlk), "bb": bbk})
    return in_maps


def kernel(x, weight, bias):
    nc = _get_program()
    in_maps = _shard_inputs(x, weight, bias)
    res = run_bass_kernel_spmd(nc, in_maps, list(range(N_CORES))).results
    out = np.empty((BATCH, OUT_DIM), np.float32)
    for k in range(N_CORES):
        out[:, k * O_PER:(k + 1) * O_PER] = res[k]["y"].T.astype(np.float32)
    return out


if __name__ == "__main__":
    rng = np.random.default_rng(0)
    x = rng.standard_normal((BATCH, OUT_DIM * DPC), dtype=np.float32)
    w = rng.standard_normal((OUT_DIM, OUT_DIM * DPC), dtype=np.float32)
    b_ = rng.standard_normal(OUT_DIM).astype(np.float32)
    out = kernel(x, w, b_)
    xb = x.reshape(BATCH, OUT_DIM, DPC)
    wb = np.stack([w[o, o * DPC: (o + 1) * DPC] for o in range(OUT_DIM)])
    exp = np.einsum("bod,od->bo", xb, wb) + b_
    rel = np.linalg.norm(out - exp) / np.linalg.norm(exp)
    print("rel err:", rel)
